# revision 12
# baseline (speedup 1.0000x reference)
"""Trainium2 Bass kernel for nn_Corr (stereo disparity correlation).

Math: reference computes, per (b,h,w):
    out = (1/(81*C)) * sum_c [ x*Sy + y*Sx ]
where Sx[w] = sum_{d=0..40} x[w+d]  (zero-padded beyond W)
      Sy[w] = sum_{d=1..40} y[w-d]  (zero-padded below 0)

Sharding: data-parallel over (batch, H/2) -> 8 cores, no communication.

Per-core pipeline (R = 128 (b,h) rows on this core). DVE and GpSimd share
SBUF ports, so the design minimizes their combined byte traffic:
  - Partition convention p = 2c + r (c = channel, r = row-half): pair u
    holds rows (u, u + R/2); uniform HBM partition stride -> one
    128-partition contiguous DMA per (tensor, group) into fp32 staging.
  - ScalarE casts fp32 -> bf16 into zero-padded buffers, pair stride
    553 = [41 zeros | 512 data]; pads are zeroed once per pool buffer.
  - The 41-zero gaps make the scan recurrence self-resetting at pair
    boundaries (state[i] telescopes to a pure window sum only if the
    first 41 elements are zero), so ONE bf16-input tensor_tensor_scan
    per (tensor, group) computes all T pairs' sliding sums:
        x: state[i] = sum buf[i+1..i+41] -> Sx[w] @ sxt[t*553 + 40 + w]
        y: state[i] = sum buf[i+1..i+40] -> Sy[w] @ syt[t*553 + w]
  - Products P1 = x*Sy on DVE, P2 = y*Sx on GpSimd (bf16, batched).
  - TensorE reduces over channels with a block-ones stationary
    (partition k = 2c+r -> output row m = u + (R/2)*(k%2)), accumulating
    32 pairs per PSUM tile; lhsT shared by the P1/P2 matmuls of a pair.
  - ScalarE copies each finished PSUM quarter -> SBUF with the 1/(81*C)
    scale; 4 output DMAs.
"""
import numpy as np

import concourse.bass as bass
import concourse.tile as tile
from concourse import bacc, mybir
from concourse.bass_utils import run_bass_kernel_spmd

N_CORES = 8
B, C, H, W = 4, 64, 256, 512
MAXD = 40
D = 2 * MAXD + 1  # 81
ROWS_PER_CORE = B * H // N_CORES  # 128
SCALE = 1.0 / (D * C)

PAD = 41
STR = 553    # [41 zeros | 512 data] per pair, both tensors
TAIL = 41    # readable zeros after the last pair (scan lookahead <= 41)

F32 = mybir.dt.float32
BF16 = mybir.dt.bfloat16
AOP = mybir.AluOpType
AF = mybir.ActivationFunctionType

BUFS = 2


def make_ones_const(n_rows: int = ROWS_PER_CORE) -> np.ndarray:
    """Z[k, 63 + (n_rows//2)*(k%2)] = 1. lhsT for pair u is Z[:, 63-u : 191-u],
    mapping partition k = 2c+r to output row m = u + (n_rows//2)*r."""
    import ml_dtypes
    z = np.zeros((128, 192), dtype=ml_dtypes.bfloat16)
    half = n_rows // 2
    z[0:128:2, 63] = 1
    z[1:128:2, 63 + half] = 1
    return z


def _groups(n_pairs):
    """(start_pair, T) list: small prologue groups for fast pipeline rampup
    and small epilogue groups for a short drain tail."""
    if n_pairs <= 8:
        return [(u, 2) for u in range(0, n_pairs, 2)]
    pro = [1, 1, 2, 4]
    epi = [4, 2, 1, 1]
    mid = n_pairs - sum(pro) - sum(epi)
    assert mid >= 0 and mid % 8 == 0
    sizes = pro + [8] * (mid // 8) + epi
    out = []
    u = 0
    for T in sizes:
        out.append((u, T))
        u += T
    return out


def build(n_rows: int = ROWS_PER_CORE):
    assert n_rows % 2 == 0
    n_pairs = n_rows // 2
    half = n_rows // 2
    qsize = 32 if n_pairs % 32 == 0 else n_pairs
    n_q = n_pairs // qsize
    groups = _groups(n_pairs)
    maxT = max(T for _, T in groups)
    blen = maxT * STR + TAIL

    nc = bacc.Bacc("TRN2", target_bir_lowering=False, debug=False,
                   num_devices=N_CORES)
    xs = nc.dram_tensor("xs", [C, n_rows, W], F32, kind="ExternalInput").ap()
    ys = nc.dram_tensor("ys", [C, n_rows, W], F32, kind="ExternalInput").ap()
    zs = nc.dram_tensor("zs", [128, 192], BF16, kind="ExternalInput").ap()
    os_ = nc.dram_tensor("os", [n_rows, W], F32, kind="ExternalOutput").ap()

    # p = 2c + r <-> h = r*half + u ; HBM offset(p, u, w) linear in p
    xs_v = xs.rearrange("c (r u) w -> (c r) u w", r=2)
    ys_v = ys.rearrange("c (r u) w -> (c r) u w", r=2)

    with tile.TileContext(nc) as tc:
        with (
            tc.tile_pool(name="const", bufs=1) as constp,
            tc.tile_pool(name="xf32", bufs=BUFS) as xf32p,
            tc.tile_pool(name="yf32", bufs=BUFS) as yf32p,
            tc.tile_pool(name="xbf", bufs=BUFS) as xbfp,
            tc.tile_pool(name="ybf", bufs=BUFS) as ybfp,
            tc.tile_pool(name="sx", bufs=BUFS) as sxp,
            tc.tile_pool(name="sy", bufs=BUFS) as syp,
            tc.tile_pool(name="prod", bufs=4) as prodp,
            tc.tile_pool(name="outp", bufs=1) as outp,
            tc.tile_pool(name="ps", bufs=1, space="PSUM") as psp,
        ):
            z_sb = constp.tile([128, 192], BF16)
            warm = constp.tile([128, 2], BF16, name="warm")
            nc.gpsimd.memset(warm[:], 0)

            out_sb = outp.tile([128, W], F32)
            # Tiny warmup activation: forces the lazy ACT_TABLE_LOAD to run
            # during ramp-in instead of delaying the first real cast
            # (out_sb cols are fully overwritten by the PSUM drains later).
            # Reads a memset const, not z_sb, so it does not wait on any DMA.
            nc.scalar.activation(out_sb[:, 0:2], warm[:], AF.Copy)
            psum_ts = [psp.tile([128, W], F32, tag=f"q{q}", name=f"psum_q{q}")
                       for q in range(n_q)]

            for gi, (u0, T) in enumerate(groups):
                # ---- one contiguous 128-partition DMA per tensor ----
                # y first: the group's first DVE op is the y-scan, so the
                # y DMA + cast are on the ramp-in critical path.
                xf = xf32p.tile([128, maxT * W], F32, tag="xf")
                yf = yf32p.tile([128, maxT * W], F32, tag="yf")
                xf3 = xf[:, 0:T * W].rearrange("p (t w) -> p t w", w=W)
                yf3 = yf[:, 0:T * W].rearrange("p (t w) -> p t w", w=W)
                if gi == 0:
                    # Ramp-in critical path: HW queues round-robin
                    # bandwidth across outstanding DMAs, so split the
                    # first y tile over 4 queues (and x over 2) to give
                    # it a larger share; the z constant goes last (it is
                    # only needed by the first matmul, much later).
                    cw = W // 4
                    for ci in range(4):
                        nc.sync.dma_start(
                            yf3[:, :, ci * cw:(ci + 1) * cw],
                            ys_v[:, u0:u0 + T, ci * cw:(ci + 1) * cw])
                    cw = W // 2
                    for ci in range(2):
                        nc.sync.dma_start(
                            xf3[:, :, ci * cw:(ci + 1) * cw],
                            xs_v[:, u0:u0 + T, ci * cw:(ci + 1) * cw])
                    nc.sync.dma_start(z_sb[:], zs)
                else:
                    nc.sync.dma_start(yf3[:], ys_v[:, u0:u0 + T, :])
                    nc.sync.dma_start(xf3[:], xs_v[:, u0:u0 + T, :])

                # ---- bf16 padded buffers; cast on ScalarE ----
                xbf = xbfp.tile([128, blen], BF16, tag="xbf")
                ybf = ybfp.tile([128, blen], BF16, tag="ybf")
                xb3 = xbf[:, 0:T * STR].rearrange("p (t q) -> p t q", q=STR)
                yb3 = ybf[:, 0:T * STR].rearrange("p (t q) -> p t q", q=STR)
                if gi < BUFS:
                    # Zero only the pad columns (plus the tail, reached as
                    # unit maxT's "pad"), once per pool buffer: pads are
                    # never overwritten afterwards (data regions are
                    # re-cast each group), so they stay zero on reuse. On
                    # GpSimd: it is otherwise idle, and the strided memset
                    # is ~12x smaller than zeroing the whole buffer, so
                    # the first group's buffers are ready early in ramp-in.
                    yb_pads = ybf[:, 0:maxT * STR].rearrange(
                        "p (t q) -> p t q", q=STR)
                    xb_pads = xbf[:, 0:maxT * STR].rearrange(
                        "p (t q) -> p t q", q=STR)
                    nc.gpsimd.memset(yb_pads[:, :, 0:PAD], 0)
                    nc.gpsimd.memset(ybf[:, maxT * STR:blen], 0)
                    nc.gpsimd.memset(xb_pads[:, :, 0:PAD], 0)
                    nc.gpsimd.memset(xbf[:, maxT * STR:blen], 0)
                nc.scalar.activation(yb3[:, :, PAD:STR], yf3[:], AF.Copy)
                nc.scalar.activation(xb3[:, :, PAD:STR], xf3[:], AF.Copy)

                # ---- batched sliding-sum scans, one per tensor (DVE) ----
                # Outputs written shifted so S*[w] of pair t lands at
                # t*STR + PAD + w, aligned with the padded data layout:
                # products become single contiguous 2-dim ops.
                L = T * STR
                sxt = sxp.tile([128, maxT * STR + TAIL], BF16, tag="sx")
                syt = syp.tile([128, maxT * STR + TAIL], BF16, tag="sy")
                # GpSimd shares SBUF ports with DVE: running it alongside
                # saturated DVE slows both to ~0.6x (measured), so ALL
                # elementwise work stays on DVE and GpSimd idles.
                # Pad positions of the products multiply against zeros (or
                # are never read by the matmuls); data positions are exactly
                # x*Sy / y*Sx. Order y-scan, P1, x-scan, P2 so the group's
                # first matmul can start after ~half the DVE work.
                p1 = prodp.tile([128, maxT * STR], BF16, tag="p1")
                p2 = prodp.tile([128, maxT * STR], BF16, tag="p2")
                nc.vector.tensor_tensor_scan(
                    syt[:, 41:41 + L], ybf[:, 40:40 + L], ybf[:, 0:L],
                    0.0, op0=AOP.add, op1=AOP.subtract)
                nc.vector.tensor_tensor(
                    p1[:, 0:L], xbf[:, 0:L], syt[:, 0:L], AOP.mult)
                nc.vector.tensor_tensor_scan(
                    sxt[:, 1:1 + L], xbf[:, 41:41 + L], xbf[:, 0:L],
                    0.0, op0=AOP.add, op1=AOP.subtract)
                nc.vector.tensor_tensor(
                    p2[:, 0:L], ybf[:, 0:L], sxt[:, 0:L], AOP.mult)

                # ---- channel reduction on TensorE ----
                for t in range(T):
                    u = u0 + t
                    q = u // qsize
                    lhs = z_sb[:, 63 - u: 191 - u]
                    o = t * STR + PAD
                    nc.tensor.matmul(psum_ts[q][:], lhs,
                                     p1[:, o:o + W],
                                     start=(u % qsize == 0), stop=False)
                    nc.tensor.matmul(psum_ts[q][:], lhs,
                                     p2[:, o:o + W],
                                     start=False, stop=(u % qsize == qsize - 1))

                    if u % qsize == qsize - 1:
                        lo = qsize * q
                        if qsize == n_pairs:  # small builds: copy everything
                            nc.scalar.activation(out_sb[:], psum_ts[q][:],
                                                 AF.Copy, scale=SCALE)
                            nc.sync.dma_start(os_[0:n_rows, :],
                                              out_sb[0:n_rows, :])
                        else:
                            nc.scalar.activation(
                                out_sb[lo:lo + qsize, :],
                                psum_ts[q][lo:lo + qsize, :],
                                AF.Copy, scale=SCALE)
                            nc.scalar.activation(
                                out_sb[half + lo:half + lo + qsize, :],
                                psum_ts[q][half + lo:half + lo + qsize, :],
                                AF.Copy, scale=SCALE)
                            nc.sync.dma_start(os_[lo:lo + qsize, :],
                                              out_sb[lo:lo + qsize, :])
                            nc.sync.dma_start(
                                os_[half + lo:half + lo + qsize, :],
                                out_sb[half + lo:half + lo + qsize, :])

    nc.compile()
    return nc


_NC_CACHE = {}


def _get_nc(n_rows=ROWS_PER_CORE):
    if n_rows not in _NC_CACHE:
        _NC_CACHE[n_rows] = build(n_rows)
    return _NC_CACHE[n_rows]


def kernel(x: np.ndarray, y: np.ndarray) -> np.ndarray:
    x = np.ascontiguousarray(np.asarray(x, dtype=np.float32))
    y = np.ascontiguousarray(np.asarray(y, dtype=np.float32))
    assert x.shape == (B, C, H, W) and y.shape == (B, C, H, W)

    nc = _get_nc()
    z = make_ones_const()
    hh = H // 2
    in_maps = []
    for k in range(N_CORES):
        b, h0 = divmod(k, 2)
        h0 *= hh
        in_maps.append({
            "xs": np.ascontiguousarray(x[b, :, h0:h0 + hh, :]),
            "ys": np.ascontiguousarray(y[b, :, h0:h0 + hh, :]),
            "zs": z,
        })
    res = run_bass_kernel_spmd(nc, in_maps, core_ids=list(range(N_CORES)))
    out = np.empty((B, H, W), dtype=np.float32)
    for k in range(N_CORES):
        b, h0 = divmod(k, 2)
        h0 *= hh
        out[b, h0:h0 + hh, :] = res.results[k]["os"]
    return out



# revision 15
# speedup vs baseline: 2.3120x; 2.3120x over previous
"""Trainium2 Bass kernel for nn_Corr (stereo disparity correlation).

Math: reference computes, per (b,h,w):
    out = (1/(81*C)) * sum_c [ x*Sy + y*Sx ]
where Sx[w] = sum_{d=0..40} x[w+d]  (zero-padded beyond W)
      Sy[w] = sum_{d=1..40} y[w-d]  (zero-padded below 0)

Sharding: data-parallel over (batch, H/2) -> 8 cores, no communication.

Per-core pipeline (R = 128 (b,h) rows on this core). DVE and GpSimd share
SBUF ports, so the design minimizes their combined byte traffic:
  - Partition convention p = 2c + r (c = channel, r = row-half): pair u
    holds rows (u, u + R/2); uniform HBM partition stride -> one
    128-partition contiguous DMA per (tensor, group) into fp32 staging.
  - ScalarE casts fp32 -> bf16 into zero-padded buffers, pair stride
    553 = [41 zeros | 512 data]; pads are zeroed once per pool buffer.
  - The 41-zero gaps make the scan recurrence self-resetting at pair
    boundaries (state[i] telescopes to a pure window sum only if the
    first 41 elements are zero), so ONE bf16-input tensor_tensor_scan
    per (tensor, group) computes all T pairs' sliding sums:
        x: state[i] = sum buf[i+1..i+41] -> Sx[w] @ sxt[t*553 + 40 + w]
        y: state[i] = sum buf[i+1..i+40] -> Sy[w] @ syt[t*553 + w]
  - Products P1 = x*Sy on DVE, P2 = y*Sx on GpSimd (bf16, batched).
  - TensorE reduces over channels with a block-ones stationary
    (partition k = 2c+r -> output row m = u + (R/2)*(k%2)), accumulating
    32 pairs per PSUM tile; lhsT shared by the P1/P2 matmuls of a pair.
  - ScalarE copies each finished PSUM quarter -> SBUF with the 1/(81*C)
    scale; 4 output DMAs.
"""
import numpy as np

import concourse.bass as bass
import concourse.tile as tile
from concourse import bacc, mybir
from concourse.bass_utils import run_bass_kernel_spmd

N_CORES = 8
B, C, H, W = 4, 64, 256, 512
MAXD = 40
D = 2 * MAXD + 1  # 81
ROWS_PER_CORE = B * H // N_CORES  # 128
SCALE = 1.0 / (D * C)

PAD = 41
STR = 553    # [41 zeros | 512 data] per pair, both tensors
TAIL = 41    # readable zeros after the last pair (scan lookahead <= 41)

F32 = mybir.dt.float32
BF16 = mybir.dt.bfloat16
AOP = mybir.AluOpType
AF = mybir.ActivationFunctionType

BUFS = 2


def make_ones_const(n_rows: int = ROWS_PER_CORE) -> np.ndarray:
    """Z[k, 63 + (n_rows//2)*(k%2)] = 1. lhsT for pair u is Z[:, 63-u : 191-u],
    mapping partition k = 2c+r to output row m = u + (n_rows//2)*r."""
    import ml_dtypes
    z = np.zeros((128, 192), dtype=ml_dtypes.bfloat16)
    half = n_rows // 2
    z[0:128:2, 63] = 1
    z[1:128:2, 63 + half] = 1
    return z


def _groups(n_pairs):
    """(start_pair, T) list: small prologue groups for fast pipeline rampup
    and small epilogue groups for a short drain tail."""
    if n_pairs <= 8:
        return [(u, 2) for u in range(0, n_pairs, 2)]
    pro = [2, 2, 4]
    epi = [4, 2, 2]
    mid = n_pairs - sum(pro) - sum(epi)
    assert mid >= 0 and mid % 8 == 0
    sizes = pro + [8] * (mid // 8) + epi
    out = []
    u = 0
    for T in sizes:
        out.append((u, T))
        u += T
    return out


def build(n_rows: int = ROWS_PER_CORE):
    assert n_rows % 2 == 0
    n_pairs = n_rows // 2
    half = n_rows // 2
    qsize = 32 if n_pairs % 32 == 0 else n_pairs
    n_q = n_pairs // qsize
    groups = _groups(n_pairs)
    maxT = max(T for _, T in groups)
    blen = maxT * STR + TAIL

    nc = bacc.Bacc("TRN2", target_bir_lowering=False, debug=False,
                   num_devices=N_CORES)
    xs = nc.dram_tensor("xs", [C, n_rows, W], F32, kind="ExternalInput").ap()
    ys = nc.dram_tensor("ys", [C, n_rows, W], F32, kind="ExternalInput").ap()
    zs = nc.dram_tensor("zs", [128, 192], BF16, kind="ExternalInput").ap()
    os_ = nc.dram_tensor("os", [n_rows, W], F32, kind="ExternalOutput").ap()

    # p = 2c + r <-> h = r*half + u ; HBM offset(p, u, w) linear in p
    xs_v = xs.rearrange("c (r u) w -> (c r) u w", r=2)
    ys_v = ys.rearrange("c (r u) w -> (c r) u w", r=2)

    with tile.TileContext(nc) as tc:
        with (
            tc.tile_pool(name="const", bufs=1) as constp,
            # bufs=1: group g+1's input DMA waits (WAR) until group g's
            # casts finish, serializing input DMAs so each gets full HBM
            # bandwidth. Steady state still hides them: DMA (11.2us for
            # T=8) + casts (4.5us) fit inside a 22us group period, and
            # ramp-in benefits most (first tile lands at full rate).
            tc.tile_pool(name="xf32", bufs=1) as xf32p,
            tc.tile_pool(name="yf32", bufs=1) as yf32p,
            tc.tile_pool(name="xbf", bufs=BUFS) as xbfp,
            tc.tile_pool(name="ybf", bufs=BUFS) as ybfp,
            tc.tile_pool(name="sx", bufs=BUFS) as sxp,
            tc.tile_pool(name="sy", bufs=BUFS) as syp,
            tc.tile_pool(name="prod", bufs=4) as prodp,
            tc.tile_pool(name="outp", bufs=1) as outp,
            tc.tile_pool(name="ps", bufs=1, space="PSUM") as psp,
        ):
            z_sb = constp.tile([128, 192], BF16)
            warm = constp.tile([128, 2], BF16, name="warm")
            nc.gpsimd.memset(warm[:], 0)

            out_sb = outp.tile([128, W], F32)
            # Tiny warmup activation: forces the lazy ACT_TABLE_LOAD to run
            # during ramp-in instead of delaying the first real cast
            # (out_sb cols are fully overwritten by the PSUM drains later).
            # Reads a memset const, not z_sb, so it does not wait on any DMA.
            nc.scalar.activation(out_sb[:, 0:2], warm[:], AF.Copy)
            psum_ts = [psp.tile([128, W], F32, tag=f"q{q}", name=f"psum_q{q}")
                       for q in range(n_q)]

            for gi, (u0, T) in enumerate(groups):
                # ---- one contiguous 128-partition DMA per tensor ----
                # y first: the group's first DVE op is the y-scan, so the
                # y DMA + cast are on the ramp-in critical path.
                xf = xf32p.tile([128, maxT * W], F32, tag="xf")
                yf = yf32p.tile([128, maxT * W], F32, tag="yf")
                xf3 = xf[:, 0:T * W].rearrange("p (t w) -> p t w", w=W)
                yf3 = yf[:, 0:T * W].rearrange("p (t w) -> p t w", w=W)
                if gi == 0:
                    # Ramp-in critical path: HW queues round-robin
                    # bandwidth across outstanding DMAs, so split the
                    # first y tile over 4 queues (and x over 2) to give
                    # it a larger share; the z constant goes last (it is
                    # only needed by the first matmul, much later).
                    cw = W // 2
                    for ci in range(2):
                        nc.sync.dma_start(
                            yf3[:, :, ci * cw:(ci + 1) * cw],
                            ys_v[:, u0:u0 + T, ci * cw:(ci + 1) * cw])
                    nc.sync.dma_start(xf3[:], xs_v[:, u0:u0 + T, :])
                    nc.sync.dma_start(z_sb[:], zs)
                else:
                    nc.sync.dma_start(yf3[:], ys_v[:, u0:u0 + T, :])
                    nc.sync.dma_start(xf3[:], xs_v[:, u0:u0 + T, :])

                # ---- bf16 padded buffers; cast on ScalarE ----
                xbf = xbfp.tile([128, blen], BF16, tag="xbf")
                ybf = ybfp.tile([128, blen], BF16, tag="ybf")
                xb3 = xbf[:, 0:T * STR].rearrange("p (t q) -> p t q", q=STR)
                yb3 = ybf[:, 0:T * STR].rearrange("p (t q) -> p t q", q=STR)
                if gi < BUFS:
                    # Zero only the pad columns (plus the tail, reached as
                    # unit maxT's "pad"), once per pool buffer: pads are
                    # never overwritten afterwards (data regions are
                    # re-cast each group), so they stay zero on reuse. On
                    # GpSimd: it is otherwise idle, and the strided memset
                    # is ~12x smaller than zeroing the whole buffer, so
                    # the first group's buffers are ready early in ramp-in.
                    yb_pads = ybf[:, 0:maxT * STR].rearrange(
                        "p (t q) -> p t q", q=STR)
                    xb_pads = xbf[:, 0:maxT * STR].rearrange(
                        "p (t q) -> p t q", q=STR)
                    nc.gpsimd.memset(yb_pads[:, :, 0:PAD], 0)
                    nc.gpsimd.memset(ybf[:, maxT * STR:blen], 0)
                    nc.gpsimd.memset(xb_pads[:, :, 0:PAD], 0)
                    nc.gpsimd.memset(xbf[:, maxT * STR:blen], 0)
                nc.scalar.activation(yb3[:, :, PAD:STR], yf3[:], AF.Copy)
                nc.scalar.activation(xb3[:, :, PAD:STR], xf3[:], AF.Copy)

                # ---- batched sliding-sum scans, one per tensor (DVE) ----
                # Outputs written shifted so S*[w] of pair t lands at
                # t*STR + PAD + w, aligned with the padded data layout:
                # products become single contiguous 2-dim ops.
                L = T * STR
                sxt = sxp.tile([128, maxT * STR + TAIL], BF16, tag="sx")
                syt = syp.tile([128, maxT * STR + TAIL], BF16, tag="sy")
                # GpSimd shares SBUF ports with DVE: running it alongside
                # saturated DVE slows both to ~0.6x (measured), so ALL
                # elementwise work stays on DVE and GpSimd idles.
                # Pad positions of the products multiply against zeros (or
                # are never read by the matmuls); data positions are exactly
                # x*Sy / y*Sx. Order y-scan, P1, x-scan, P2 so the group's
                # first matmul can start after ~half the DVE work.
                p1 = prodp.tile([128, maxT * STR], BF16, tag="p1")
                p2 = prodp.tile([128, maxT * STR], BF16, tag="p2")
                nc.vector.tensor_tensor_scan(
                    syt[:, 41:41 + L], ybf[:, 40:40 + L], ybf[:, 0:L],
                    0.0, op0=AOP.add, op1=AOP.subtract)
                nc.vector.tensor_tensor(
                    p1[:, 0:L], xbf[:, 0:L], syt[:, 0:L], AOP.mult)
                nc.vector.tensor_tensor_scan(
                    sxt[:, 1:1 + L], xbf[:, 41:41 + L], xbf[:, 0:L],
                    0.0, op0=AOP.add, op1=AOP.subtract)
                nc.vector.tensor_tensor(
                    p2[:, 0:L], ybf[:, 0:L], sxt[:, 0:L], AOP.mult)

                # ---- channel reduction on TensorE ----
                for t in range(T):
                    u = u0 + t
                    q = u // qsize
                    lhs = z_sb[:, 63 - u: 191 - u]
                    o = t * STR + PAD
                    nc.tensor.matmul(psum_ts[q][:], lhs,
                                     p1[:, o:o + W],
                                     start=(u % qsize == 0), stop=False)
                    nc.tensor.matmul(psum_ts[q][:], lhs,
                                     p2[:, o:o + W],
                                     start=False, stop=(u % qsize == qsize - 1))

                    if u % qsize == qsize - 1:
                        lo = qsize * q
                        if qsize == n_pairs:  # small builds: copy everything
                            nc.scalar.activation(out_sb[:], psum_ts[q][:],
                                                 AF.Copy, scale=SCALE)
                            nc.sync.dma_start(os_[0:n_rows, :],
                                              out_sb[0:n_rows, :])
                        else:
                            nc.scalar.activation(
                                out_sb[lo:lo + qsize, :],
                                psum_ts[q][lo:lo + qsize, :],
                                AF.Copy, scale=SCALE)
                            nc.scalar.activation(
                                out_sb[half + lo:half + lo + qsize, :],
                                psum_ts[q][half + lo:half + lo + qsize, :],
                                AF.Copy, scale=SCALE)
                            nc.sync.dma_start(os_[lo:lo + qsize, :],
                                              out_sb[lo:lo + qsize, :])
                            nc.sync.dma_start(
                                os_[half + lo:half + lo + qsize, :],
                                out_sb[half + lo:half + lo + qsize, :])

    nc.compile()
    return nc


_NC_CACHE = {}


def _get_nc(n_rows=ROWS_PER_CORE):
    if n_rows not in _NC_CACHE:
        _NC_CACHE[n_rows] = build(n_rows)
    return _NC_CACHE[n_rows]


def kernel(x: np.ndarray, y: np.ndarray) -> np.ndarray:
    x = np.ascontiguousarray(np.asarray(x, dtype=np.float32))
    y = np.ascontiguousarray(np.asarray(y, dtype=np.float32))
    assert x.shape == (B, C, H, W) and y.shape == (B, C, H, W)

    nc = _get_nc()
    z = make_ones_const()
    hh = H // 2
    in_maps = []
    for k in range(N_CORES):
        b, h0 = divmod(k, 2)
        h0 *= hh
        in_maps.append({
            "xs": np.ascontiguousarray(x[b, :, h0:h0 + hh, :]),
            "ys": np.ascontiguousarray(y[b, :, h0:h0 + hh, :]),
            "zs": z,
        })
    res = run_bass_kernel_spmd(nc, in_maps, core_ids=list(range(N_CORES)))
    out = np.empty((B, H, W), dtype=np.float32)
    for k in range(N_CORES):
        b, h0 = divmod(k, 2)
        h0 *= hh
        out[b, h0:h0 + hh, :] = res.results[k]["os"]
    return out

